# revision 1
# baseline (speedup 1.0000x reference)
"""[vt: bf16 es/vext, bf16, per-quarter proj interleave] Causal self-attention (B=4, L=2048, D=1024, H=16) on 8 Trainium2 NeuronCores.

Sharding: core c -> (batch b = c//2, head-group g = c%2 of 8 heads).
Each core computes qkv projection for its 8 heads, causal attention, and a
partial out-projection (its head-group's rows of W_out). The host sums the
two partials per batch and adds biases.

v2 over baseline:
  - Diagonal trimming: for the 4 diagonal k-tiles of each q-block, columns
    [0 : o*128) are entirely above the causal boundary -> the scores matmul,
    exp, mask and AV accumulation all operate on [o*128 : 512) only.
  - PSUM evacuations (posb, qk copies, yb) pinned to DVE so ACT runs ~pure exp.
  - Single shared PSUM matmul pool (3 x [128,1024] slots) + po0/po1 accum
    banks = 8 banks, letting projection / attention / out-proj matmuls
    coexist; emission order proj(h0), qb0, qb1, proj(h1), qb2, qb3 overlaps
    half-1 projection (PE-heavy) with first-half attention (ACT-heavy).
  - qT is per-q-block rotating (kT/vext persist) to fit SBUF.

All matmuls run as float32r (fp32 data, reduced-precision PE mode).
Attention layout (transpose-free):
  qT, kT   [64d x L]  per head (2 heads stacked per 128 partitions)
  S^T tile [128k x 512q] = kT_tile.T @ qT_block   (PE, K=64, row-packed pair)
  expS     = exp(S^T)  (ACT, PSUM->SBUF), causal-masked on diagonal tiles
  O^T,sums [65 x 512q] += [V_tile | ones].T-form @ expS  (PE, K=128)
  O^T_norm = O^T * broadcast(1/sums)  -> directly the lhsT of out-proj
  Y tile   [128l x 1024e] = sum_pairs O^T_pair.T @ Wo_pair
"""

import os
from contextlib import ExitStack

import numpy as np

os.environ.setdefault("JAX_PLATFORMS", "")

import concourse.bass as bass
import concourse.mybir as mybir
import concourse.tile as tile
from concourse import bacc, bass_utils

F32 = mybir.dt.float32
F32R = mybir.dt.float32r
BF16 = mybir.dt.bfloat16
AF = mybir.ActivationFunctionType

B, L, D, H = 4, 2048, 1024, 16
DK = D // H            # 64
G = 2                  # head groups (tensor parallel)
HPG = H // G           # 8 heads per group
GW = HPG * DK          # 512 columns per group
P = 128
CO = D // P            # 8 contraction tiles for projections
LT = L // P            # 16 l-tiles / k-tiles
QW = 512               # q-block width
QB = L // QW           # 4 q-blocks
NPAIR = HPG // 2       # 4 head-pairs per group (2 heads per 128 partitions)

_NC_CACHE: dict = {}


def build_nc(with_qk_bias: bool, repeat: int = 1, **_ignored):
    nc = bacc.Bacc("TRN2", target_bir_lowering=False, debug=False, num_devices=8)

    xt = nc.dram_tensor("xt", [D, L], BF16, kind="ExternalInput").ap()
    wq = nc.dram_tensor("wq", [D, GW], BF16, kind="ExternalInput").ap()
    wk = nc.dram_tensor("wk", [D, GW], BF16, kind="ExternalInput").ap()
    wv = nc.dram_tensor("wv", [D, GW], BF16, kind="ExternalInput").ap()
    wo = nc.dram_tensor("wo", [GW, D], F32, kind="ExternalInput").ap()
    # mband[k, t] = 1.0 iff t >= k (the 128x128 diagonal triangle); col P = 1s
    mband = nc.dram_tensor("mband", [P, P + 1], F32, kind="ExternalInput").ap()
    if with_qk_bias:
        bq = nc.dram_tensor("bq", [P, NPAIR], F32, kind="ExternalInput").ap()
        bk = nc.dram_tensor("bk", [P, NPAIR], F32, kind="ExternalInput").ap()
    y = nc.dram_tensor("y", [L, D], F32, kind="ExternalOutput").ap()

    xt_r = xt.rearrange("(co p) l -> co p l", p=P)
    wq_r = wq.rearrange("(co p) c -> co p c", p=P)
    wk_r = wk.rearrange("(co p) c -> co p c", p=P)
    wv_r = wv.rearrange("(co p) c -> co p c", p=P)
    wo_r = wo.rearrange("(pr p) e -> pr p e", p=P)
    y_r = y.rearrange("(lt p) e -> lt p e", p=P)

    def mm(out, lhsT, rhs, start, stop, **kw):
        nc.tensor.matmul(out, lhsT, rhs, start=start, stop=stop, **kw)

    with tile.TileContext(nc) as tc, ExitStack() as ctx:
        constp = ctx.enter_context(tc.tile_pool(name="const", bufs=1))
        mband_sb = constp.tile([P, P + 1], F32)
        if with_qk_bias:
            bq_sb = constp.tile([P, NPAIR], F32)
            bk_sb = constp.tile([P, NPAIR], F32)
            nc.sync.dma_start(bq_sb[:], bq)
            nc.sync.dma_start(bk_sb[:], bk)
        else:
            bq_sb = bk_sb = None

        # persistent SBUF
        kp = ctx.enter_context(tc.tile_pool(name="k", bufs=1))
        kT = kp.tile([P, NPAIR, L], F32R)        # 32 KB
        vp = ctx.enter_context(tc.tile_pool(name="v", bufs=1))
        vext = vp.tile([P, LT, HPG, DK + 1], BF16)  # 36 KB; last col = ones
        wop = ctx.enter_context(tc.tile_pool(name="wo", bufs=1))
        wo_sb = wop.tile([P, NPAIR, D], F32R)    # 16 KB

        # rotating pools
        qp = ctx.enter_context(tc.tile_pool(name="q", bufs=3))
        xtp = ctx.enter_context(tc.tile_pool(name="xt", bufs=2))
        wp = ctx.enter_context(tc.tile_pool(name="w", bufs=6))
        esp = ctx.enter_context(tc.tile_pool(name="es", bufs=6))
        otp = ctx.enter_context(tc.tile_pool(name="ot", bufs=3))    # 24 KB
        rcp = ctx.enter_context(tc.tile_pool(name="rc", bufs=3))
        ybp = ctx.enter_context(tc.tile_pool(name="yb", bufs=3))
        pmm = ctx.enter_context(tc.tile_pool(name="pmm", bufs=2, space="PSUM"))
        pfp = ctx.enter_context(tc.tile_pool(name="pf", bufs=1, space="PSUM"))
        pop = ctx.enter_context(tc.tile_pool(name="po", bufs=1, space="PSUM"))

        env = dict(locals())
        for _rep in range(repeat):
            _kernel_body(nc, tc, mm, with_qk_bias, env)

    nc.compile()
    return nc


def _kernel_body(nc, tc, mm, with_qk_bias, env):
    (kT, vext, wo_sb, mband_sb, bq_sb, bk_sb) = (
        env["kT"], env["vext"], env["wo_sb"], env["mband_sb"],
        env["bq_sb"], env["bk_sb"])
    (qp, xtp, wp, esp, otp, rcp, ybp, pmm, pfp, pop) = (
        env["qp"], env["xtp"], env["wp"], env["esp"], env["otp"], env["rcp"],
        env["ybp"], env["pmm"], env["pfp"], env["pop"])
    (xt_r, wq_r, wk_r, wv_r, wo_r, y_r) = (
        env["xt_r"], env["wq_r"], env["wk_r"], env["wv_r"], env["wo_r"],
        env["y_r"])

    qT_blk = [None] * QB  # per-q-block qT tiles, filled by the projections
    _pfc = [0]

    def next_pf():
        # alternate filler matmul groups between the dedicated filler slot
        # and the attention-scores pool; slot grants resolve by priority.
        _pfc[0] += 1
        pool, tag = ((pfp, "pf"), (pmm, "mm"))[_pfc[0] % 2]
        return pool.tile([P, 2 * QW], F32, tag=tag, name=f"pf{_pfc[0]}")

    def proj_xt(lh):
        # xt on the Pool DMA queue, in 2-co chunks, so it streams in
        # parallel with the weight loads on the SP queue.
        lbase = lh * (L // 2)
        xq = []
        chunks = [(0, 2), (2, 2), (4, 2), (6, 2)]
        for q in range(2):
            xt_sb = xtp.tile([P, CO, QW], BF16, tag="xt")
            for c, n in chunks:
                nc.gpsimd.dma_start(
                    xt_sb[:, c:c + n],
                    xt_r[c:c + n, :, lbase + q * QW:lbase + (q + 1) * QW]
                    .transpose((1, 0, 2)))
            xq.append(xt_sb)
        return xq

    def load_w_half(w_dram, h):
        w_sb = wp.tile([P, CO, GW // 2], BF16, tag="w")
        for c in range(0, CO, 2):
            nc.sync.dma_start(
                w_sb[:, c:c + 2],
                w_dram[c:c + 2, :, h * (GW // 2):(h + 1) * (GW // 2)]
                .transpose((1, 0, 2)))
        return w_sb

    def proj_piece(lh, xq, q, wh_q, wh_k, wh_v):
        """q/k/v projection for one quarter (q-block lh*2+q)."""
        for wh, is_q in ((wh_q, True), (wh_k, False)):
            qb = lh * 2 + q
            if is_q:
                blk = qp.tile([P, NPAIR, QW], F32R, tag="q")
                qT_blk[qb] = blk
            for pp in range(2):
                pt = next_pf()
                for u in range(2):
                    pair = 2 * pp + u
                    for co in range(CO):
                        mm(pt[:, u * QW:(u + 1) * QW],
                           wh[pp][:, co, u * P:(u + 1) * P],
                           xq[q][:, co],
                           start=co == 0, stop=co == CO - 1)
                for u in range(2):
                    pair = 2 * pp + u
                    src = pt[:, u * QW:(u + 1) * QW]
                    if is_q:
                        dst = qT_blk[qb][:, pair]
                    else:
                        dst = kT[:, pair, qb * QW:(qb + 1) * QW]
                    if with_qk_bias:
                        bt = bq_sb if is_q else bk_sb
                        nc.vector.tensor_scalar_add(
                            dst, src, bt[:, pair:pair + 1])
                    elif lh == 0:
                        nc.scalar.copy(dst, src)
                    else:
                        nc.vector.tensor_copy(dst, src)

        # v projection for this quarter's 4 l-tiles
        lbase = lh * (L // 2)
        wh = wh_v
        for lt2 in range(2):
            pv = next_pf()
            for u in range(2):
                lt = (lh * 2 + q) * 4 + 2 * lt2 + u
                loc = lt * P - lbase
                for h in range(2):
                    for co in range(CO):
                        mm(pv[:, u * QW + h * P * 2:u * QW + (h + 1) * P * 2],
                           xq[loc // QW][:, co, loc % QW:loc % QW + P],
                           wh[h][:, co],
                           start=co == 0, stop=co == CO - 1)
            for u in range(2):
                lt = (lh * 2 + q) * 4 + 2 * lt2 + u
                nc.vector.tensor_copy(
                    vext[:, lt, :, 0:DK],
                    pv[:, u * QW:(u + 1) * QW].rearrange(
                        "p (h d) -> p h d", h=HPG))
                nc.vector.tensor_copy(
                    vext[:, lt, :, DK:DK + 1],
                    mband_sb[:, P:P + 1, None].to_broadcast((P, HPG, 1)))


    def out_proj_part(oT, qb, i):
        lt = 4 * qb + i
        yb = ybp.tile([P, D], F32, tag="yb")
        # qb3 parts can only run after all attention (their oT finalizes
        # last), so they may use the attention-scores PSUM slots freely;
        # earlier parts act as mid-attention fillers and must keep to the
        # dedicated filler slot to not stall the scores rotation.
        py = next_pf() if qb == 3 else pfp.tile([P, 2 * QW], F32, tag="pf")
        for eh in range(2):
            for pair in range(NPAIR):
                mm(py[:, eh * QW:(eh + 1) * QW],
                   oT[:, pair, i * P:(i + 1) * P],
                   wo_sb[:, pair, eh * QW:(eh + 1) * QW],
                   start=pair == 0, stop=pair == NPAIR - 1)
            nc.vector.tensor_copy(yb[:, eh * QW:(eh + 1) * QW],
                                  py[:, eh * QW:(eh + 1) * QW])
            nc.sync.dma_start(y_r[lt, :, eh * QW:(eh + 1) * QW],
                              yb[:, eh * QW:(eh + 1) * QW])

    oT_blk = [None] * QB

    def attn_qb(qb):
        nj = 4 * qb + 4          # number of valid k-tiles
        oT = otp.tile([P, NPAIR, QW], F32R, tag="ot")
        for pair in range(NPAIR):
            po0 = pop.tile([DK + 1, QW], F32, tag="po0", name="po0")
            po1 = pop.tile([DK + 1, QW], F32, tag="po1", name="po1")
            for j in range(nj):
                o = j - 4 * qb           # diagonal index (>=0 on diagonal)
                lo = o * P if o > 0 else 0   # first causally-valid column
                wv_ = QW - lo
                ps2 = pmm.tile([P, 2 * QW], F32, tag="mm")
                es2 = esp.tile([P, 2 * QW], BF16, tag="es")
                qs0 = qT_blk[qb][0:DK, pair, lo:QW]
                qs1 = qT_blk[qb][DK:P, pair, lo:QW]
                nc.tensor.matmul(ps2[:, lo:QW],
                                 kT[0:DK, pair, j * P:(j + 1) * P],
                                 qs0, start=True, stop=True,
                                 tile_position=(0, 0))
                nc.tensor.matmul(ps2[:, QW + lo:2 * QW],
                                 kT[DK:P, pair, j * P:(j + 1) * P],
                                 qs1, start=True, stop=True,
                                 tile_position=(64, 0))
                if lo == 0:
                    nc.scalar.activation(es2[:], ps2[:], AF.Exp)
                else:
                    er = es2.rearrange("p (u q) -> p u q", u=2)[:, :, lo:QW]
                    pr_ = ps2.rearrange("p (u q) -> p u q", u=2)[:, :, lo:QW]
                    nc.scalar.activation(er, pr_, AF.Exp)
                if o >= 0:  # triangle mask on the 128-wide diagonal block
                    dj = es2.rearrange("p (u q) -> p u q", u=2)[:, :, lo:lo + P]
                    nc.vector.tensor_mul(
                        dj, dj,
                        mband_sb[:, None, 0:P].to_broadcast((P, 2, P)))
                mm(po0[:, lo:QW], vext[:, j, 2 * pair, :], es2[:, lo:QW],
                   start=j == 0, stop=j == nj - 1)
                mm(po1[:, lo:QW], vext[:, j, 2 * pair + 1, :],
                   es2[:, QW + lo:2 * QW],
                   start=j == 0, stop=j == nj - 1)
            for u, po in ((0, po0), (1, po1)):
                # evacuate [O; sums] in one copy (frees the po bank for the
                # next pair's AV fast), realign sums to partition 0 for the
                # approx reciprocal (HW quirk), normalize from SBUF. The
                # copies run on ACT in the first-half window (DVE is the
                # congested engine there) and on DVE later (ACT-bound region).
                posb = rcp.tile([DK + 1, QW], F32, tag="posb")
                nc.vector.tensor_copy(posb[:], po[:])
                sm = rcp.tile([1, QW], F32, tag="sm")
                nc.vector.tensor_copy(sm[:], posb[DK:DK + 1, :])
                rc = rcp.tile([1, QW], F32, tag="rc")
                rcb = rcp.tile([DK, QW], F32, tag="rcb")
                nc.vector.reciprocal_approx_fast(rc[:], sm[:])
                nc.gpsimd.partition_broadcast(rcb[:], rc[:])
                nc.vector.tensor_mul(
                    oT[u * DK:(u + 1) * DK, pair, :], posb[0:DK, :], rcb[:])
            # qb0's out-projection interleaves into qb1; later q-blocks'
            # out-projections are emitted last (lowest priority) so the
            # scheduler uses them as PE gap filler during qb2/qb3.
            if qb == 1:
                out_proj_part(oT_blk[0], 0, pair)
        oT_blk[qb] = oT

    xq0 = proj_xt(0)
    wh_q = [load_w_half(wq_r, h) for h in range(2)]
    wh_k = [load_w_half(wk_r, h) for h in range(2)]
    # mband (mask + ones col) must be emitted before proj_piece's
    # ones-copies read it; here it is off the startup DMA critical path.
    nc.sync.dma_start(mband_sb[:], env["mband"])
    wh_v = [load_w_half(wv_r, h) for h in range(2)]
    proj_piece(0, xq0, 0, wh_q, wh_k, wh_v)
    for pr in range(0, NPAIR, 2):
        nc.sync.dma_start(wo_sb[:, pr:pr + 2],
                          wo_r[pr:pr + 2].transpose((1, 0, 2)).bitcast(F32R))
    attn_qb(0)
    proj_piece(0, xq0, 1, wh_q, wh_k, wh_v)
    xq1 = proj_xt(1)
    attn_qb(1)
    wh_q = [load_w_half(wq_r, h) for h in range(2)]
    wh_k = [load_w_half(wk_r, h) for h in range(2)]
    wh_v = [load_w_half(wv_r, h) for h in range(2)]
    proj_piece(1, xq1, 0, wh_q, wh_k, wh_v)
    attn_qb(2)
    proj_piece(1, xq1, 1, wh_q, wh_k, wh_v)
    attn_qb(3)
    for qb in (1, 2, 3):
        for i in range(4):
            out_proj_part(oT_blk[qb], qb, i)


def _prep_inputs(x, W_qkv, b_qkv, W_out):
    """Per-core input maps. Core c -> batch c//2, head-group c%2."""
    x = np.ascontiguousarray(np.asarray(x, dtype=np.float32))
    W_qkv = np.asarray(W_qkv, dtype=np.float32)
    b_qkv = np.asarray(b_qkv, dtype=np.float32)
    W_out = np.asarray(W_out, dtype=np.float32)

    scale = 1.0 / np.sqrt(DK)
    mband = np.concatenate(
        [(np.arange(P)[None, :] >= np.arange(P)[:, None]).astype(np.float32),
         np.ones((P, 1), dtype=np.float32)], axis=1)

    with_qk_bias = bool(np.any(b_qkv[:2 * D]))
    xts = [np.ascontiguousarray(x[b].T) for b in range(B)]
    in_maps = []
    for c in range(8):
        b, g = c // 2, c % 2
        sl = slice(g * GW, (g + 1) * GW)
        import ml_dtypes
        bf = ml_dtypes.bfloat16
        m = {
            "xt": xts[b].astype(bf),
            "wq": (np.ascontiguousarray(W_qkv[:, g * GW:(g + 1) * GW]) * scale).astype(bf),
            "wk": np.ascontiguousarray(W_qkv[:, D + g * GW:D + (g + 1) * GW]).astype(bf),
            "wv": np.ascontiguousarray(W_qkv[:, 2 * D + g * GW:2 * D + (g + 1) * GW]).astype(bf),
            "wo": np.ascontiguousarray(W_out[sl, :]),
            "mband": mband,
        }
        if with_qk_bias:
            m["bq"] = np.ascontiguousarray(
                b_qkv[g * GW:(g + 1) * GW].reshape(NPAIR, P).T) * scale
            m["bk"] = np.ascontiguousarray(
                b_qkv[D + g * GW:D + (g + 1) * GW].reshape(NPAIR, P).T)
        in_maps.append(m)
    return in_maps, with_qk_bias


def kernel(x, W_qkv, b_qkv, W_out, b_out):
    in_maps, with_qk_bias = _prep_inputs(x, W_qkv, b_qkv, W_out)

    key = ("nc", with_qk_bias)
    if key not in _NC_CACHE:
        _NC_CACHE[key] = build_nc(with_qk_bias)
    nc = _NC_CACHE[key]

    res = bass_utils.run_bass_kernel_spmd(nc, in_maps, core_ids=list(range(8)))
    parts = [r["y"] for r in res.results]

    b_qkv = np.asarray(b_qkv, dtype=np.float32)
    W_out_np = np.asarray(W_out, dtype=np.float32)
    corr = (b_qkv[2 * D:3 * D] @ W_out_np
            + np.asarray(b_out, dtype=np.float32)).astype(np.float32)

    out = np.empty((B, L, D), dtype=np.float32)
    for b in range(B):
        out[b] = parts[2 * b] + parts[2 * b + 1] + corr
    return out

